# revision 13
# baseline (speedup 1.0000x reference)
"""Trainium2 Bass kernel for nn_AutoencoderDecoderLayer (S=1024, B=8, E=1024, NH=16, F=4096).

Strategy: data-parallel over batch B=8 -> one batch element per NeuronCore,
no collectives. Per core one full decoder layer over (S=1024, E=1024) tokens.

All matmuls run in fp16 (same PE rate as bf16, ~8x less rounding error) with
fp32 PSUM accumulation; residual/normalization arithmetic is fp32.

Layout choices (host pre-transposes weights so every DMA is contiguous):
  - activations transposed (feature-on-partition) act as matmul lhsT
  - weights W.T (in, out) act as matmul rhs
  - attention scores computed transposed: scoresT[tj, ti] = k_h^T q_h so the
    softmax numerator exp() feeds the AV matmul as lhsT with no transpose
  - softmax skips max-subtraction (scores ~ N(0,1); exp(s-4) is fp16-safe)
    and gets its denominator from an appended ones-column on V
"""

import sys

sys.path.insert(0, "/opt/trn_rl_repo")

from contextlib import ExitStack

import numpy as np

import concourse.bass as bass
import concourse.mybir as mybir
import concourse.tile as tile
from concourse.masks import make_identity
from concourse.vector_clock import ScopedClock

P = 128
S, B, E, NH, F = 1024, 8, 1024, 16, 4096
HD = E // NH  # 64
TT = S // P  # 8 token tiles
KC = E // P  # 8 contraction chunks over E
ZK = 9  # contraction chunks over E+1 (bias row), padded to 1152
FBLK = 4  # f blocks of 1024
FT_PER_B = 8  # f tiles per block
EXP_SHIFT = -4.0  # uniform shift inside exp(); cancels in softmax normalize

f32 = mybir.dt.float32
f16 = mybir.dt.float16

_MAX_DRAIN_WAITS = 1


def _split_drain_and_barrier(self, tick_clock, wait_clock):
    """This walrus build rejects >1 sem-wait on a CTRL Drain; split the final
    tile drain's wait list across a chain of Drains on the same engine."""
    drain_inst = self.nc.sync.drain()
    wait_clock.add_sem_waits(
        drain_inst.ins, ScopedClock({None: tick_clock.global_clock})
    )
    si = drain_inst.ins.sync_info
    if si is not None and len(si.on_wait) > _MAX_DRAIN_WAITS:
        waits = list(si.on_wait)
        drain_inst.ins.sync_info = mybir.SyncInfo(
            on_wait=waits[:_MAX_DRAIN_WAITS], on_update=list(si.on_update)
        )
        rest = waits[_MAX_DRAIN_WAITS:]
        for i in range(0, len(rest), _MAX_DRAIN_WAITS):
            extra = self.nc.sync.drain()
            extra.ins.sync_info = mybir.SyncInfo(
                on_wait=rest[i : i + _MAX_DRAIN_WAITS], on_update=[]
            )
    self.nc.all_engine_barrier()
    assert self.sems is not None
    popped = self.nc._tile_sem_poison_stack.pop()
    assert popped is self._sem_poison
    self.nc.clear_and_free_semaphores(list(self.sems.allocated().values()))
    self.nc.all_engine_barrier()


tile.TileContext._drain_and_barrier = _split_drain_and_barrier


def _split_waits_in_bir(bir_bytes):
    """This walrus build accepts at most ONE sem-wait per instruction.
    Hoist extra on_wait entries onto NoOp instructions inserted just before
    the owning instruction on the same engine (waits AND together, and each
    engine executes its stream in order, so this is semantics-preserving)."""
    import json

    d = json.loads(bir_bytes)
    cnt = 0

    def fix_block(blk):
        nonlocal cnt
        insts = blk.get("instructions") or []
        out = []
        for ins in insts:
            si = ins.get("sync_info")
            if si:
                waits = si.get("on_wait") or []
                if len(waits) > 1:
                    for w in waits[:-1]:
                        cnt += 1
                        out.append(
                            {
                                "name": f"wsplit-{cnt}",
                                "opcode": "NoOp",
                                "engine": ins["engine"],
                                "ins": [],
                                "outs": [],
                                "sync_info": {"on_wait": [w], "on_update": []},
                            }
                        )
                    si["on_wait"] = waits[-1:]
            out.append(ins)
        blk["instructions"] = out
        for sub in blk.get("blocks") or []:
            fix_block(sub)

    for fn in d.get("functions", []):
        for b in fn.get("blocks", []):
            fix_block(b)
    return json.dumps(d).encode()


def _install_bir_wait_split():
    from concourse import bass2jax, bass_utils

    if getattr(bass_utils, "_orig_compile_bir_kernel", None) is None:
        bass_utils._orig_compile_bir_kernel = bass_utils.compile_bir_kernel

        def patched(bir_json, tmpdir, neff_name="file.neff"):
            return bass_utils._orig_compile_bir_kernel(
                _split_waits_in_bir(bir_json), tmpdir, neff_name=neff_name
            )

        bass_utils.compile_bir_kernel = patched
        bass2jax.compile_bir_kernel = patched


_install_bir_wait_split()


def build_program():
    nc = bass.Bass("TRN2", target_bir_lowering=False, debug=False, num_devices=1)

    def din(name, shape, dt):
        return nc.dram_tensor(name, shape, dt, kind="ExternalInput").ap()

    xT = din("xT", (E, S), f16)
    xr = din("xr", (S, E), f32)
    wqT = din("wqT", (E, E), f16)
    wkT = din("wkT", (E, E), f16)
    wvT = din("wvT", (E, E), f16)
    woT = din("woT", (E, E), f16)
    pghT = din("pghT", (E, E), f16)
    fc1T = din("fc1T", (E, F), f16)
    fc2T = din("fc2T", (F, E), f16)
    pgzTb = din("pgzTb", (ZK * P, E), f16)
    pvTb = din("pvTb", (ZK * P, E), f16)
    zpad = din("zpad", (ZK * P,), f32)
    bqs_d = din("bqs", (E,), f32)  # pre-scaled by 1/sqrt(HD)
    bks_d = din("bks", (E,), f32)
    bv_d = din("bvv", (E,), f16)
    bo_d = din("bob", (E,), f16)
    fc1b_d = din("fc1b", (F,), f32)
    fc2b_d = din("fc2b", (E,), f16)
    lng_d = [din(n, (E,), f16) for n in ("g1", "bb1", "g2", "bb2", "g3", "bb3")]
    cmask_d = din("cmask", (P, P), f32)
    out = nc.dram_tensor("out", (S, E), f32, kind="ExternalOutput").ap()

    with tile.TileContext(nc) as tc, ExitStack() as top:
        pool = lambda st, nm, bufs, **kw: st.enter_context(
            tc.tile_pool(name=nm, bufs=bufs, **kw)
        )
        # Long-lived pools go on the LEFT allocation stack (released at the
        # end, in reverse entry order); phase-scoped pools nest on the RIGHT
        # stack so their SBUF is reclaimed between phases (strict LIFO).
        const = pool(top, "const", 1, side="left")
        wpool = pool(top, "wpool", 10, side="left")
        tmpp = pool(top, "tmpp", 2, side="left")
        smallp = pool(top, "smallp", 8, side="left")
        psum = pool(top, "psum", 1, space="PSUM")

        def ps512(nm):
            return psum.tile([P, 512], f32, tag="mm512", bufs=4, name=nm)

        def ps65(nm):
            return psum.tile([P, 65], f32, tag="av65", bufs=2, name=nm)

        def pstr(nm, dt=f32):
            return psum.tile([P, P], dt, tag="tr128", bufs=2, name=nm)

        # ---------------- constants ----------------
        ident16 = const.tile([P, P], f16, name="ident16")
        make_identity(nc, ident16)
        ident32 = const.tile([P, P], f32, name="ident32")
        make_identity(nc, ident32)
        cmask = const.tile([P, P], f32, name="cmask_sb")
        nc.sync.dma_start(cmask, cmask_d)
        eps_t = const.tile([P, 1], f32, name="eps_t")
        nc.vector.memset(eps_t, 1e-5)
        expshift_t = const.tile([P, 1], f32, name="expshift_t")
        nc.vector.memset(expshift_t, EXP_SHIFT)
        bqs = const.tile([P, KC], f32, name="bqs_sb")
        nc.sync.dma_start(bqs, bqs_d.rearrange("(o p) -> p o", p=P))
        bks = const.tile([P, KC], f32, name="bks_sb")
        nc.sync.dma_start(bks, bks_d.rearrange("(o p) -> p o", p=P))
        fc1bs = const.tile([P, F // P], f32, name="fc1bs_sb")
        nc.sync.dma_start(fc1bs, fc1b_d.rearrange("(o p) -> p o", p=P))

        def bcast_const(name, dvec):
            t = const.tile([P, E], f16, name=name)
            nc.sync.dma_start(t, dvec[None, :].to_broadcast([P, E]))
            return t

        bv_bc = bcast_const("bv_bc", bv_d)
        bo_bc = bcast_const("bo_bc", bo_d)
        fc2b_bc = bcast_const("fc2b_bc", fc2b_d)
        g1_bc = bcast_const("g1_bc", lng_d[0])
        b1_bc = bcast_const("b1_bc", lng_d[1])
        g2_bc = bcast_const("g2_bc", lng_d[2])
        b2_bc = bcast_const("b2_bc", lng_d[3])
        g3_bc = bcast_const("g3_bc", lng_d[4])
        b3_bc = bcast_const("b3_bc", lng_d[5])

        zsb = const.tile([P, ZK], f32, name="zsb")
        nc.sync.dma_start(zsb, zpad.rearrange("(o p) -> p o", p=P))
        zrep = const.tile([P, ZK, P], f16, name="zrep")
        for k in range(ZK):
            nc.vector.tensor_copy(
                out=zrep[:, k, :], in_=zsb[:, k : k + 1].to_broadcast([P, P])
            )

        def load_w_tiles(src, n, tag="w", pool_=None, cols=None):
            pool_ = pool_ or wpool
            tiles = []
            for kc in range(n):
                w = cols[1] - cols[0] if cols else src.shape[1]
                t = pool_.tile([P, w], f16, tag=tag, name=f"w_{src.tensor.name}_{kc}")
                if cols:
                    nc.sync.dma_start(t, src[kc * P : (kc + 1) * P, cols[0] : cols[1]])
                else:
                    nc.sync.dma_start(t, src[kc * P : (kc + 1) * P, :])
                tiles.append(t)
            return tiles

        # ---------------- layernorm helper (in place, fp32) ----------------
        def layer_norm_inplace(t, g_bc, b_bc, nm):
            stats = smallp.tile([P, 2, 6], f32, tag="stats", name=f"st_{nm}")
            for sg in range(2):
                nc.vector.bn_stats(
                    out=stats[:, sg, :], in_=t[:, sg * 512 : (sg + 1) * 512]
                )
            mv = smallp.tile([P, 2], f32, tag="mv", name=f"mv_{nm}")
            nc.vector.bn_aggr(out=mv, in_=stats)
            sd = smallp.tile([P, 1], f32, tag="sd", name=f"sd_{nm}")
            nc.scalar.activation(
                sd, mv[:, 1:2], mybir.ActivationFunctionType.Sqrt, bias=eps_t, scale=1.0
            )
            rstd = smallp.tile([P, 1], f32, tag="rstd", name=f"rs_{nm}")
            nc.vector.reciprocal(rstd, sd)
            nc.vector.tensor_scalar(
                t,
                t,
                scalar1=mv[:, 0:1],
                scalar2=rstd,
                op0=mybir.AluOpType.subtract,
                op1=mybir.AluOpType.mult,
            )
            nc.vector.tensor_tensor(t, t, g_bc, mybir.AluOpType.mult)
            nc.vector.tensor_tensor(t, t, b_bc, mybir.AluOpType.add)

        def transpose_to_f16(src_tiles, pool_, tag, npfx):
            outs = []
            for et in range(KC):
                o = pool_.tile([P, S], f16, tag=tag, name=f"{npfx}_{et}")
                for tt in range(TT):
                    pt = pstr(f"tr{npfx}{et}_{tt}")
                    nc.tensor.transpose(
                        pt, src_tiles[tt][:, et * P : (et + 1) * P], ident32
                    )
                    nc.scalar.activation(
                        o[:, tt * P : (tt + 1) * P],
                        pt,
                        mybir.ActivationFunctionType.Copy,
                    )
                outs.append(o)
            return outs

        res = []

        with ExitStack() as blk1:
            lnT1p = pool(blk1, "lnT1p", TT, side="right")
            attn_outer = blk1.enter_context(ExitStack())
            attnTp = pool(attn_outer, "attnTp", TT, side="right")
            with ExitStack() as attn_scope:
                qkp = pool(attn_scope, "qkp", 2 * TT, side="right")
                v1p = pool(attn_scope, "v1p", TT, side="right")
                expp = pool(attn_scope, "expp", 8, side="right")
                attnp = pool(attn_scope, "attnp", TT, side="right")

                with ExitStack() as x_scope:
                    xTp = pool(x_scope, "xTp", TT, side="right")
                    xTs = []
                    for kc in range(KC):
                        t = xTp.tile([P, S], f16, tag="xT", name=f"xT_{kc}")
                        nc.sync.dma_start(t, xT[kc * P : (kc + 1) * P, :])
                        xTs.append(t)

                    # ---- q/k (transposed layout) ----
                    def proj_T(wtiles, bias_cols, scale, tag, namepfx):
                        outs = []
                        for et in range(KC):
                            pss = [ps512(f"{namepfx}_ps{et}_{j}") for j in range(2)]
                            for kc in range(KC):
                                for j in range(2):
                                    nc.tensor.matmul(
                                        pss[j],
                                        wtiles[kc][:, et * P : (et + 1) * P],
                                        xTs[kc][:, j * 512 : (j + 1) * 512],
                                        start=(kc == 0),
                                        stop=(kc == KC - 1),
                                    )
                            o = qkp.tile([P, S], f16, tag=tag, name=f"{namepfx}_{et}")
                            for j in range(2):
                                nc.scalar.activation(
                                    o[:, j * 512 : (j + 1) * 512],
                                    pss[j],
                                    mybir.ActivationFunctionType.Identity,
                                    bias=bias_cols[:, et : et + 1],
                                    scale=scale,
                                )
                            outs.append(o)
                        return outs

                    qTs = proj_T(
                        load_w_tiles(wqT, KC), bqs, 1.0 / float(np.sqrt(HD)), "qk", "qT"
                    )
                    kTs = proj_T(load_w_tiles(wkT, KC), bks, 1.0, "qk", "kT")

                    # ---- v (token-major) + ones column ----
                    wv_tiles = load_w_tiles(wvT, KC)
                    v1s = []
                    for tt in range(TT):
                        pss = [ps512(f"v_ps{tt}_{j}") for j in range(2)]
                        for kc in range(KC):
                            for j in range(2):
                                nc.tensor.matmul(
                                    pss[j],
                                    xTs[kc][:, tt * P : (tt + 1) * P],
                                    wv_tiles[kc][:, j * 512 : (j + 1) * 512],
                                    start=(kc == 0),
                                    stop=(kc == KC - 1),
                                )
                        v1 = v1p.tile([P, NH, HD + 1], f16, tag="v1", name=f"v1_{tt}")
                        for j in range(2):
                            nc.vector.tensor_tensor(
                                v1[:, j * 8 : (j + 1) * 8, 0:HD],
                                pss[j].rearrange("p (h d) -> p h d", d=HD),
                                bv_bc[:, j * 512 : (j + 1) * 512].rearrange(
                                    "p (h d) -> p h d", d=HD
                                ),
                                mybir.AluOpType.add,
                            )
                        nc.vector.memset(v1[:, :, HD : HD + 1], 1.0)
                        v1s.append(v1)

                # ---- attention (per head) ----
                attns = [
                    attnp.tile([P, E], f16, tag="attn", name=f"attn_{tt}")
                    for tt in range(TT)
                ]
                for h in range(NH):
                    qh = qTs[h // 2][(h % 2) * HD : (h % 2) * HD + HD, :]
                    kh = kTs[h // 2][(h % 2) * HD : (h % 2) * HD + HD, :]
                    exps = []
                    for tjt in range(TT):
                        ex = expp.tile([P, S], f16, tag="exp", name=f"exp_{h}_{tjt}")
                        exps.append(ex)
                        base = tjt * P
                        off = base
                        while off < S:
                            n = min(512, S - off)
                            ps = ps512(f"s_ps{h}_{tjt}_{off}")
                            nc.tensor.matmul(
                                ps[:, :n],
                                kh[:, base : base + P],
                                qh[:, off : off + n],
                                start=True,
                                stop=True,
                            )
                            if off == base:
                                nc.vector.tensor_tensor(
                                    ps[:, 0:P], ps[:, 0:P], cmask, mybir.AluOpType.add
                                )
                            nc.scalar.activation(
                                ex[:, off : off + n],
                                ps[:, :n],
                                mybir.ActivationFunctionType.Exp,
                                bias=expshift_t,
                                scale=1.0,
                            )
                            off += n
                    for tit in range(TT):
                        pav = ps65(f"av{h}_{tit}")
                        for tjt in range(tit + 1):
                            nc.tensor.matmul(
                                pav,
                                exps[tjt][:, tit * P : (tit + 1) * P],
                                v1s[tjt][:, h, :],
                                start=(tjt == 0),
                                stop=(tjt == tit),
                            )
                        rc = smallp.tile([P, 1], f32, tag="rc", name=f"rc{h}_{tit}")
                        nc.vector.reciprocal(rc, pav[:, HD : HD + 1])
                        nc.vector.tensor_scalar_mul(
                            attns[tit][:, h * HD : (h + 1) * HD], pav[:, 0:HD], rc
                        )

                # ---- transpose attn -> attnT ----
                attnTs = []
                for et in range(KC):
                    at = attnTp.tile([P, S], f16, tag="attnT", name=f"attnT_{et}")
                    for tt in range(TT):
                        pt = pstr(f"trA{et}_{tt}", f16)
                        nc.tensor.transpose(
                            pt, attns[tt][:, et * P : (et + 1) * P], ident16
                        )
                        nc.scalar.activation(
                            at[:, tt * P : (tt + 1) * P],
                            pt,
                            mybir.ActivationFunctionType.Copy,
                        )
                    attnTs.append(at)
            # attention pools closed here

            # ---- wo projection + residual + LN1 ----
            resp = pool(top, "resp", TT, side="left")
            wo_tiles = load_w_tiles(woT, KC)
            for tt in range(TT):
                pss = [ps512(f"o_ps{tt}_{j}") for j in range(2)]
                for kc in range(KC):
                    for j in range(2):
                        nc.tensor.matmul(
                            pss[j],
                            attnTs[kc][:, tt * P : (tt + 1) * P],
                            wo_tiles[kc][:, j * 512 : (j + 1) * 512],
                            start=(kc == 0),
                            stop=(kc == KC - 1),
                        )
                xr_t = tmpp.tile([P, E], f32, tag="xr", name=f"xr_{tt}")
                nc.sync.dma_start(xr_t, xr[tt * P : (tt + 1) * P, :])
                r = resp.tile([P, E], f32, tag="res", name=f"res_{tt}")
                for j in range(2):
                    nc.vector.tensor_tensor(
                        r[:, j * 512 : (j + 1) * 512],
                        pss[j],
                        xr_t[:, j * 512 : (j + 1) * 512],
                        mybir.AluOpType.add,
                    )
                nc.vector.tensor_tensor(r, r, bo_bc, mybir.AluOpType.add)
                layer_norm_inplace(r, g1_bc, b1_bc, f"ln1_{tt}")
                res.append(r)

            attn_outer.close()  # release attnTp

            ln1Ts = transpose_to_f16(res, lnT1p, "lnT1", "ln1T")

            # ---- z projections (broadcast over tokens) ----
            with ExitStack() as z_scope:
                zwpool = pool(z_scope, "zwpool", ZK, side="right")
                zbcp = pool(top, "zbcp", 2, side="left")

                def z_proj(wsrc, nm):
                    ztiles = load_w_tiles(wsrc, ZK, tag="wz", pool_=zwpool)
                    pss = [ps512(f"{nm}_ps{j}") for j in range(2)]
                    for kc in range(ZK):
                        for j in range(2):
                            nc.tensor.matmul(
                                pss[j],
                                zrep[:, kc, :],
                                ztiles[kc][:, j * 512 : (j + 1) * 512],
                                start=(kc == 0),
                                stop=(kc == ZK - 1),
                            )
                    o = zbcp.tile([P, E], f32, tag="zbc", name=nm)
                    for j in range(2):
                        nc.scalar.activation(
                            o[:, j * 512 : (j + 1) * 512],
                            pss[j],
                            mybir.ActivationFunctionType.Copy,
                        )
                    return o

                zg_bc = z_proj(pgzTb, "zg_bc")
                zv_bc = z_proj(pvTb, "zv_bc")

            # ---- gated fusion + LN2 ----
            pgh_tiles = load_w_tiles(pghT, KC)
            for tt in range(TT):
                pss = [ps512(f"g_ps{tt}_{j}") for j in range(2)]
                for kc in range(KC):
                    for j in range(2):
                        nc.tensor.matmul(
                            pss[j],
                            ln1Ts[kc][:, tt * P : (tt + 1) * P],
                            pgh_tiles[kc][:, j * 512 : (j + 1) * 512],
                            start=(kc == 0),
                            stop=(kc == KC - 1),
                        )
                gt = tmpp.tile([P, E], f32, tag="gate", name=f"gate_{tt}")
                for j in range(2):
                    nc.vector.tensor_tensor(
                        gt[:, j * 512 : (j + 1) * 512],
                        pss[j],
                        zg_bc[:, j * 512 : (j + 1) * 512],
                        mybir.AluOpType.add,
                    )
                nc.scalar.activation(gt, gt, mybir.ActivationFunctionType.Sigmoid)
                nc.vector.tensor_tensor(gt, gt, zv_bc, mybir.AluOpType.mult)
                nc.vector.tensor_tensor(res[tt], res[tt], gt, mybir.AluOpType.add)
                layer_norm_inplace(res[tt], g2_bc, b2_bc, f"ln2_{tt}")
        # attnTp, lnT1p, zbcp closed here

        # ---- FFN (f-blocked), accumulate into res ----
        with ExitStack() as ffn_scope:
            lnT2p = pool(ffn_scope, "lnT2p", TT, side="right")
            hTp = pool(ffn_scope, "hTp", FT_PER_B + 4, side="right")
            ln2Ts = transpose_to_f16(res, lnT2p, "lnT2", "ln2T")
            for tt in range(TT):
                nc.vector.tensor_tensor(
                    res[tt], res[tt], fc2b_bc, mybir.AluOpType.add
                )
            for fb in range(FBLK):
                f1tiles = load_w_tiles(fc1T, KC, cols=(fb * 1024, (fb + 1) * 1024))
                f2tiles = []
                for i in range(FT_PER_B):
                    t = wpool.tile([P, E], f16, tag="w", name=f"fc2w_{fb}_{i}")
                    gr = (fb * FT_PER_B + i) * P
                    nc.sync.dma_start(t, fc2T[gr : gr + P, :])
                    f2tiles.append(t)
                hts = []
                for ftl in range(FT_PER_B):
                    pss = [ps512(f"h_ps{fb}_{ftl}_{j}") for j in range(2)]
                    for kc in range(KC):
                        for j in range(2):
                            nc.tensor.matmul(
                                pss[j],
                                f1tiles[kc][:, ftl * P : (ftl + 1) * P],
                                ln2Ts[kc][:, j * 512 : (j + 1) * 512],
                                start=(kc == 0),
                                stop=(kc == KC - 1),
                            )
                    ht = hTp.tile([P, S], f16, tag="hT", name=f"hT_{fb}_{ftl}")
                    ft = fb * FT_PER_B + ftl
                    for j in range(2):
                        nc.scalar.activation(
                            ht[:, j * 512 : (j + 1) * 512],
                            pss[j],
                            mybir.ActivationFunctionType.Relu,
                            bias=fc1bs[:, ft : ft + 1],
                            scale=1.0,
                        )
                    hts.append(ht)
                for tt in range(TT):
                    pss = [ps512(f"y_ps{fb}_{tt}_{j}") for j in range(2)]
                    for i in range(FT_PER_B):
                        for j in range(2):
                            nc.tensor.matmul(
                                pss[j],
                                hts[i][:, tt * P : (tt + 1) * P],
                                f2tiles[i][:, j * 512 : (j + 1) * 512],
                                start=(i == 0),
                                stop=(i == FT_PER_B - 1),
                            )
                    for j in range(2):
                        nc.vector.tensor_tensor(
                            res[tt][:, j * 512 : (j + 1) * 512],
                            res[tt][:, j * 512 : (j + 1) * 512],
                            pss[j],
                            mybir.AluOpType.add,
                        )

        # ---- LN3 + store ----
        for tt in range(TT):
            layer_norm_inplace(res[tt], g3_bc, b3_bc, f"ln3_{tt}")
            nc.sync.dma_start(out[tt * P : (tt + 1) * P, :], res[tt])

    return nc


def prep_inputs(inputs):
    """Shard the full inputs into 8 per-core in_maps (core b <- batch b)."""
    f16c = lambda a: np.ascontiguousarray(np.asarray(a), dtype=np.float16)
    f32c = lambda a: np.ascontiguousarray(np.asarray(a), dtype=np.float32)

    x = np.asarray(inputs["x"], np.float32)  # (S, B, E)
    z = np.asarray(inputs["z"], np.float32)  # (1, B, E)

    shared = {
        "wqT": f16c(np.asarray(inputs["wq"]).T),
        "wkT": f16c(np.asarray(inputs["wk"]).T),
        "wvT": f16c(np.asarray(inputs["wv"]).T),
        "woT": f16c(np.asarray(inputs["wo"]).T),
        "pghT": f16c(np.asarray(inputs["pgh_w"]).T),
        "fc1T": f16c(np.asarray(inputs["fc1_w"]).T),
        "fc2T": f16c(np.asarray(inputs["fc2_w"]).T),
        "bqs": f32c(np.asarray(inputs["bq"]) / np.sqrt(HD)),
        "bks": f32c(inputs["bk"]),
        "bvv": f16c(inputs["bv"]),
        "bob": f16c(inputs["bo"]),
        "fc1b": f32c(inputs["fc1_b"]),
        "fc2b": f16c(inputs["fc2_b"]),
        "g1": f16c(inputs["ln1_g"]),
        "bb1": f16c(inputs["ln1_b"]),
        "g2": f16c(inputs["ln2_g"]),
        "bb2": f16c(inputs["ln2_b"]),
        "g3": f16c(inputs["ln3_g"]),
        "bb3": f16c(inputs["ln3_b"]),
    }
    pgzTb = np.zeros((ZK * P, E), np.float16)
    pgzTb[:E] = f16c(np.asarray(inputs["pgz_w"]).T)
    pgzTb[E] = f16c(np.asarray(inputs["pgz_b"]) + np.asarray(inputs["pgh_b"]))
    shared["pgzTb"] = pgzTb
    pvTb = np.zeros((ZK * P, E), np.float16)
    pvTb[:E] = f16c(np.asarray(inputs["pv_w"]).T)
    pvTb[E] = f16c(inputs["pv_b"])
    shared["pvTb"] = pvTb

    ti = np.arange(P)
    shared["cmask"] = np.where(ti[None, :] >= ti[:, None], 0.0, -1e9).astype(np.float32)

    in_maps = []
    for b in range(B):
        xb = x[:, b, :]
        zp = np.zeros((ZK * P,), np.float32)
        zp[:E] = z[0, b]
        zp[E] = 1.0
        m = dict(shared)
        m["xT"] = f16c(xb.T)
        m["xr"] = f32c(xb)
        m["zpad"] = zp
        in_maps.append(m)
    return in_maps


_NC_CACHE = None


def get_program():
    global _NC_CACHE
    if _NC_CACHE is None:
        _NC_CACHE = build_program()
    return _NC_CACHE


def kernel(**inputs):
    from concourse.bass_utils import run_bass_kernel_spmd

    nc = get_program()
    in_maps = prep_inputs(inputs)
    res = run_bass_kernel_spmd(nc, in_maps, core_ids=list(range(B)))
    return np.stack([res.results[b]["out"] for b in range(B)], axis=1)
